# revision 38
# baseline (speedup 1.0000x reference)
"""Trainium2 Bass kernel: transformer block (biased attention + residual).

Reference math (B=4, S=1024, H=1024, NH=16, DK=64):
    q = x_q @ Wq.T ; k = x_kv @ Wk.T ; v = x_kv @ Wv.T   (per-head reshape)
    scores = q k^T / sqrt(DK) + bias ; attn = softmax(scores)
    out = x_q + (attn v reshaped) @ Wo.T

Sharding: 8 cores = 4 batches x 2 head-groups (8 heads each). Each core
computes its (batch, head-group) slice; the host sums the partial outputs
per batch and adds the residual.

Per-core dataflow:
    qT/kT = W_g x^T (bf16 PE), cast to fp8 on PSUM drain.
    scoresT[k,q] = one fp8 DoubleRow matmul per tile contracting BOTH
        (k_h zero-padded to 128 rows) x (q) and (identity) x (bias fp8):
        qk/8 + bias lands in PSUM in 256 PE cycles per [128,512].
    expT = exp(scores - 5.5) on ACT over [128,1024] PSUM, fp8 out (shift
        keeps fp8e4 in range; it cancels in softmax normalization).
    avT  = fp8 DoubleRow (v_aug two-mt-tile packed) x expT -> 65 rows:
        rows 0..63 attn-out^T, row 64 = softmax denominator.
    aoT  = av[0:64] * broadcast(1/denom)  (K=1 ones matmul + DVE mult)
    yA   = Wo chunks 0-2 contraction (bf16), yB = chunk 3; host adds.
"""

import sys

import numpy as np

for _p in ("/opt/trn_rl_repo",):
    if _p not in sys.path:
        sys.path.append(_p)

B, S, H, NH = 4, 1024, 1024, 16
DK = 64
P = 128
NH_L = 8            # heads per core
JL = NH_L * DK      # 512 local head dims per core
FT = H // P         # 8 contraction tiles for projections
TT = S // P         # 8 seq (k-position) tiles
JC = JL // P        # 4 local head-dim chunks of 128
QF = 512            # matmul moving free dim (one PSUM bank of fp32)
QC = S // QF        # 2 q chunks
N_CORES = 8
NSLOT = NH_L * TT   # 64 k-tile slots; slot NSLOT is the identity
BR = 4              # bias ring pair slots per head-pair tile (must be even)


def _split_waits(nc, max_waits=1):
    """This walrus build rejects instructions carrying more than ~1 sem
    wait ("Too many sync wait commands" in setupSyncWait). Hoist surplus
    waits onto same-engine NoOps spliced immediately before the carrying
    instruction — same engine position, so semantics are unchanged."""
    import bass_rust
    import concourse.mybir as mybir

    n = 0
    for f in nc.m.functions:
        for bb in f.blocks:
            new_insts = []
            for inst in bb.instructions:
                si = inst.sync_info
                waits = list(si.on_wait) if si and si.on_wait else []
                if len(waits) > max_waits:
                    keep = waits[:max_waits]
                    extra = waits[max_waits:]
                    for i in range(0, len(extra), max_waits):
                        nop = mybir.InstNoOp(name=f"WSPLIT-{n}", ins=[], outs=[])
                        n += 1
                        nop.engine = inst.engine
                        nop.bass_nofuse = False
                        nop.debug = inst.debug
                        nop.sync_info = bass_rust.SyncInfo(
                            on_wait=extra[i : i + max_waits], on_update=[]
                        )
                        new_insts.append(nop)
                    si.on_wait = keep
                    inst.sync_info = si
                new_insts.append(inst)
            bb.instructions[:] = new_insts


_prog = None


def _build():
    global _prog
    if _prog is not None:
        return _prog

    import concourse.bass as bass
    import concourse.mybir as mybir
    import concourse.tile as tile
    from concourse.masks import make_identity

    f32 = mybir.dt.float32
    bf16 = mybir.dt.bfloat16
    fp8 = mybir.dt.float8e4
    EXP = mybir.ActivationFunctionType.Exp
    MULT = mybir.AluOpType.mult
    DR = mybir.MatmulPerfMode.DoubleRow

    nc = bass.Bass()
    # x / W projections travel as fp8 (value, residual) pairs; the two
    # DoubleRow passes x8.w8 + rx8.w8 + x8.rw8 recover near-bf16 accuracy.
    xqT_d = nc.declare_dram_parameter("xqT", [2, H, S], fp8, isOutput=False)
    xkvT_d = nc.declare_dram_parameter("xkvT", [2, H, S], fp8, isOutput=False)
    wqT_d = nc.declare_dram_parameter("wqT", [2, H, JL], fp8, isOutput=False)
    wkT_d = nc.declare_dram_parameter("wkT", [2, H, JL], fp8, isOutput=False)
    wvT_d = nc.declare_dram_parameter("wvT", [2, H, JL], fp8, isOutput=False)
    woT_d = nc.declare_dram_parameter("woT", [JL, H], bf16, isOutput=False)
    biasT_d = nc.declare_dram_parameter(
        "biasT", [NH_L, TT, 2, P, S], fp8, isOutput=False
    )
    yA_d = nc.declare_dram_parameter("yA", [H, S], bf16, isOutput=True)
    yB_d = nc.declare_dram_parameter("yB", [H, S], bf16, isOutput=True)

    with tile.TileContext(nc) as tc:
        with (
            nc.allow_low_precision(reason="fp8 attention pipeline, validated"),
            tc.tile_pool(name="singles", bufs=1) as singles,
            tc.tile_pool(name="smallp", bufs=4) as smallp,
            tc.tile_pool(name="outp", bufs=6) as outp,
            tc.tile_pool(name="ps_s", bufs=2, space="PSUM") as ps_s,
            tc.tile_pool(name="ps_mm", bufs=2, space="PSUM") as ps_mm,
            tc.tile_pool(name="ps_av", bufs=2, space="PSUM") as ps_av,
        ):
            xq_sb = singles.tile([P, 2, FT, S], fp8)
            xkv_sb = singles.tile([P, 2, FT, S], fp8)
            wq_sb = singles.tile([P, 2, FT, JL], fp8)
            wk_sb = singles.tile([P, 2, FT, JL], fp8)
            wv_sb = singles.tile([P, 2, FT, JL], fp8)
            wo_sb = singles.tile([P, JC, H], bf16)
            # KM: k-tile slots (zero-padded to 128 rows) + identity at slot 64
            KM = singles.tile([P, NSLOT + 1, P], fp8)
            # QB[hp]: slot 0 = qT(head pair hp), slot 1 = q fp8-residual,
            # slots 2..17 = (bias, bias-residual) ring pairs
            QBs = [singles.tile([P, 2 + 2 * BR, S], fp8, name=f"QB{hp}")
                   for hp in range(JC)]
            ET = singles.tile([P, NH_L, TT, S], fp8)
            v_sb = singles.tile([P, TT, NH_L, DK + 2], fp8)
            aoT_sb = singles.tile([P, JC, S], bf16)
            ident_bf = singles.tile([P, P], bf16)
            ones64 = singles.tile([1, DK], bf16)
            nbias = singles.tile([P, 1], f32)

            # -- one-time SBUF init (Pool + DVE, overlaps input DMAs).
            # Identity first so the PE warm-up can start immediately; the
            # big KM zero-pad memsets go on Pool after the xq first-half
            # load (they only gate the k-slot scatter copies).
            make_identity(nc, ident_bf)
            nc.vector.tensor_copy(out=KM[:, NSLOT, :], in_=ident_bf)
            nc.vector.memset(v_sb[:, :, :, DK : DK + 2], 1.0)
            nc.vector.memset(ones64, 1.0)
            nc.vector.memset(nbias, -5.5)

            def load2(sb, dr, cols=None, eng=None):
                drr = dr.rearrange("r (n p) j -> p r n j", p=P)
                e = eng or nc.sync
                for r in range(2):
                    for f2 in range(FT // 2):
                        s = slice(2 * f2, 2 * f2 + 2)
                        if cols is None:
                            e.dma_start(out=sb[:, r, s, :], in_=drr[:, r, s, :])
                        else:
                            e.dma_start(
                                out=sb[:, r, s, cols], in_=drr[:, r, s, cols]
                            )

            # startup queues, ordered for the critical path to exp(0,0):
            #   SP:   wq, xkv-h1, bias(0,0..1), xkv-h2, wv, wo
            #   ACT:  wk only (its exp stream starts ~10us in)
            #   Pool: xq-h1, xq-h2, KM odd-slot zero-pad
            #   DVE:  KM even-slot zero-pad (gates only the h0 k-scatter)
            km_z = KM[:, 0:NSLOT, :].rearrange(
                "p (h e) c -> p h (e c)", h=NH_L
            )
            load2(wq_sb, wqT_d)
            load2(xq_sb, xqT_d, cols=slice(0, QF), eng=nc.gpsimd)
            load2(xkv_sb, xkvT_d, cols=slice(0, QF))
            load2(xq_sb, xqT_d, cols=slice(QF, S), eng=nc.gpsimd)
            load2(wk_sb, wkT_d, eng=nc.scalar)
            nc.vector.memset(km_z[DK:P, 0:NH_L:2, :], 0.0)
            nc.gpsimd.memset(km_z[0:DK, 1:NH_L:2, :], 0.0)

            # -- bias prefetch machinery (fp8 bias + fp8 residual pairs,
            # straight into QB ring slot pairs) --
            bias_seen = set()

            def bias_fetch(h, mt, eng):
                # head-major runs: heads of a pair never run concurrently,
                # so each head cycles through all BR ring pair-slots
                if (h, mt) in bias_seen:
                    return
                bias_seen.add((h, mt))
                hp, r = h // 2, 2 * (mt % BR) + 2
                eng.dma_start(
                    out=QBs[hp][:, r : r + 2, :],
                    in_=biasT_d[h, mt].rearrange("t p s -> p t s"),
                )

            for _mt in range(3):
                bias_fetch(0, _mt, nc.sync)
            load2(xkv_sb, xkvT_d, cols=slice(QF, S))

            # wv / wo chunks dribble out on SP during runs 0-1 (needed
            # only by fillers from run 1 / run 6 onward)
            wvr = wvT_d.rearrange("r (n p) j -> p r n j", p=P)
            late_dmas = [
                (lambda r=r, f2=f2: nc.sync.dma_start(
                    out=wv_sb[:, r, 2 * f2 : 2 * f2 + 2, :],
                    in_=wvr[:, r, 2 * f2 : 2 * f2 + 2, :]))
                for r in range(2) for f2 in range(FT // 2)
            ] + [
                (lambda hdt=hdt: nc.sync.dma_start(
                    out=wo_sb[:, hdt, :],
                    in_=woT_d[hdt * P : (hdt + 1) * P, :]))
                for hdt in range(JC)
            ]

            # (HAM warm-up) back-to-back tiny matmuls while the input DMAs
            # land, so the PE clock is at 8/8 when real work starts.
            warm_ps = ps_mm.tile([P, P], f32, name="warm", tag="mm")
            for _ in range(40):
                nc.tensor.matmul(warm_ps, lhsT=ident_bf, rhs=ident_bf,
                                 start=True, stop=True, skip_group_check=True)

            def proj_qk_mms(ps, w_sb, x_sb, jcols, xcols):
                # 12 DoubleRow insts: x8.w8 + rx8.w8 + x8.rw8 over 4 ft-pairs
                for f2 in range(FT // 2):
                    s = slice(2 * f2, 2 * f2 + 2)
                    for which in range(3):
                        wr, xr = [(0, 0), (0, 1), (1, 0)][which]
                        nc.tensor.matmul(
                            ps,
                            lhsT=w_sb[:, wr, s, jcols],
                            rhs=x_sb[:, xr, s, xcols],
                            start=(f2 == 0 and which == 0),
                            stop=(f2 == FT // 2 - 1 and which == 2),
                            perf_mode=DR,
                        )

            def proj_q_unit(jc, tch):
                ps = ps_mm.tile([P, QF], f32, name=f"pjq_{jc}_{tch}", tag="mm")
                cols = slice(tch * QF, (tch + 1) * QF)
                proj_qk_mms(ps, wq_sb, xq_sb,
                            slice(jc * P, (jc + 1) * P), cols)
                nc.vector.tensor_scalar_mul(QBs[jc][:, 0, cols], ps, 1 / 64)
                # fp8 residual of the q quantization: rq8 = fp8(q/64 - q8)
                nc.vector.scalar_tensor_tensor(
                    out=QBs[jc][:, 1, cols],
                    in0=ps,
                    scalar=1 / 64,
                    in1=QBs[jc][:, 0, cols],
                    op0=MULT,
                    op1=mybir.AluOpType.subtract,
                )

            def proj_k_unit(jc, tch):
                ps = ps_mm.tile([P, QF], f32, name=f"pjk_{jc}_{tch}", tag="mm")
                proj_qk_mms(ps, wk_sb, xkv_sb,
                            slice(jc * P, (jc + 1) * P),
                            slice(tch * QF, (tch + 1) * QF))
                # scatter the two heads' k rows into their padded KM slots
                for i in range(2):
                    h = 2 * jc + i
                    rows = slice((h % 2) * DK, (h % 2) * DK + DK)
                    nc.vector.tensor_scalar_mul(
                        KM[rows, h * TT + 4 * tch : h * TT + 4 * tch + 4, :],
                        ps[rows, :].rearrange("p (m c) -> p m c", m=4),
                        1 / 64,
                    )

            def proj_v_unit(tt):
                # 8 DoubleRow insts: x8.wv8 + rx8.wv8 over 4 ft-pairs
                ps = ps_mm.tile([P, QF], f32, name=f"pjv_{tt}", tag="mm")
                kcols = slice(tt * P, (tt + 1) * P)
                for f2 in range(FT // 2):
                    s = slice(2 * f2, 2 * f2 + 2)
                    for xr in range(2):
                        nc.tensor.matmul(
                            ps,
                            lhsT=xkv_sb[:, xr, s, kcols],
                            rhs=wv_sb[:, 0, s, :],
                            start=(f2 == 0 and xr == 0),
                            stop=(f2 == FT // 2 - 1 and xr == 1),
                            perf_mode=DR,
                        )
                nc.vector.tensor_scalar_mul(
                    v_sb[:, tt, :, 0:DK],
                    ps.rearrange("p (h d) -> p h d", h=NH_L),
                    1 / 64,
                )

            def scores_step(h, mt):
                hp, s0 = h // 2, h * TT + mt
                r = 2 * (mt % BR) + 2
                QB = QBs[hp]
                lhsT = KM[:, s0 : NSLOT + 1 : NSLOT - s0, :]
                ps = ps_s.tile([P, S], f32, name=f"sc_{h}_{mt}", tag="sc")
                for qc in range(QC):
                    cols = slice(qc * QF, (qc + 1) * QF)
                    # inst1: k8.q8 + ident.b8 ; inst2: k8.rq8 + ident.rb8
                    nc.tensor.matmul(
                        ps[:, cols],
                        lhsT=lhsT,
                        rhs=QB[:, 0 : r + 1 : r, cols],
                        start=True,
                        stop=False,
                        perf_mode=DR,
                        skip_group_check=True,
                    )
                    nc.tensor.matmul(
                        ps[:, cols],
                        lhsT=lhsT,
                        rhs=QB[:, 1 : r + 2 : r, cols],
                        start=False,
                        stop=True,
                        perf_mode=DR,
                        skip_group_check=True,
                    )
                nc.scalar.activation(
                    out=ET[:, h, mt, :], in_=ps, func=EXP, bias=nbias
                )

            av_tiles = {}
            rec_tiles = {}

            def attn_v_A(h, qc, borrow_sc=False):
                if borrow_sc:
                    # tail: borrow the idle scores-PSUM ring (same tag ->
                    # same buffers), sliced down to the av shape
                    av = ps_s.tile([P, S], f32, name=f"av_{h}_{qc}",
                                   tag="sc")[0 : DK + 2, 0:QF]
                else:
                    av = ps_av.tile([DK + 2, QF], f32, name=f"av_{h}_{qc}",
                                    tag="av")
                av_tiles[(h, qc)] = av
                for mtp in range(TT // 2):
                    nc.tensor.matmul(
                        av,
                        lhsT=v_sb[:, 2 * mtp : 2 * mtp + 2, h, :],
                        rhs=ET[:, h, 2 * mtp : 2 * mtp + 2,
                               qc * QF : (qc + 1) * QF],
                        start=(mtp == 0),
                        stop=(mtp == TT // 2 - 1),
                        perf_mode=DR,
                    )
                rec = smallp.tile([1, QF], bf16, name=f"rec_{h}_{qc}", tag="rec")
                nc.vector.reciprocal(out=rec, in_=av[DK : DK + 1, :])
                rec_tiles[(h, qc)] = rec

            def attn_v_B(h, qc):
                av = av_tiles.pop((h, qc))
                bc = ps_mm.tile([DK, QF], f32, name=f"bc_{h}_{qc}", tag="mm")
                nc.tensor.matmul(
                    bc, lhsT=ones64, rhs=rec_tiles.pop((h, qc)),
                    start=True, stop=True,
                )
                # walrus allows at most one PSUM operand per TensorTensor:
                # stage the broadcast reciprocal through SBUF
                bcs = smallp.tile([DK, QF], bf16, name=f"bcs_{h}_{qc}", tag="bcs")
                nc.vector.tensor_copy(out=bcs, in_=bc)
                nc.vector.tensor_tensor(
                    out=aoT_sb[
                        (h % 2) * DK : (h % 2) * DK + DK,
                        h // 2,
                        qc * QF : (qc + 1) * QF,
                    ],
                    in0=av[0:DK, :],
                    in1=bcs,
                    op=MULT,
                )

            def y_unit(oc, qc, part, act_copy=False, borrow_sc=False):
                hd = range(JC - 1) if part == 0 else range(JC - 1, JC)
                out_d = yA_d if part == 0 else yB_d
                if borrow_sc:
                    ps = ps_s.tile([P, S], f32, name=f"y{part}_{oc}_{qc}",
                                   tag="sc")[:, 0:QF]
                else:
                    ps = ps_mm.tile([P, QF], f32, name=f"y{part}_{oc}_{qc}",
                                    tag="mm")
                first, last = hd[0], hd[-1]
                for hdt in hd:
                    nc.tensor.matmul(
                        ps,
                        lhsT=wo_sb[:, hdt, oc * P : (oc + 1) * P],
                        rhs=aoT_sb[:, hdt, qc * QF : (qc + 1) * QF],
                        start=(hdt == first),
                        stop=(hdt == last),
                    )
                ysb = outp.tile([P, QF], bf16, name=f"ysb{part}_{oc}_{qc}", tag="y")
                if act_copy:
                    nc.scalar.copy(out=ysb, in_=ps)
                else:
                    nc.vector.tensor_copy(out=ysb, in_=ps)
                (nc.sync if act_copy else nc.gpsimd).dma_start(
                    out=out_d[oc * P : (oc + 1) * P, qc * QF : (qc + 1) * QF],
                    in_=ysb,
                )

            # ---- head-major emission schedule: 8 runs of 8 scores steps;
            # each head's attn.v starts right after its own run, so only
            # head 7's chain + the yB drain remain in the tail ----
            for tch in range(QC):
                proj_q_unit(0, tch)
            proj_k_unit(0, 0)

            def av_fillers(h):
                return [
                    lambda: attn_v_A(h, 0),
                    lambda: attn_v_A(h, 1),
                    lambda: attn_v_B(h, 0),
                    lambda: attn_v_B(h, 1),
                ]

            def qk_fillers(jc):
                return (
                    [lambda t=t: proj_q_unit(jc, t) for t in range(QC)]
                    + [lambda t=t: proj_k_unit(jc, t) for t in range(QC)]
                )

            yA = [
                lambda oc=oc, qc=qc: y_unit(oc, qc, 0)
                for oc in range(FT)
                for qc in range(QC)
            ]
            vps = [lambda tt=tt: proj_v_unit(tt) for tt in range(TT)]

            fillers_by_run = [
                [lambda: proj_k_unit(0, 1)] + qk_fillers(1),
                vps[0:4] + qk_fillers(2)[0:2],
                vps[4:8] + qk_fillers(2)[2:4] + av_fillers(0)[0:3],
                av_fillers(0)[3:] + av_fillers(1) + qk_fillers(3)[0:2],
                qk_fillers(3)[2:4] + av_fillers(2) + av_fillers(3)[0:2],
                av_fillers(3)[2:] + av_fillers(4),
                av_fillers(5) + yA[0:6],
                av_fillers(6) + yA[6:16],
            ]
            for h in range(NH_L):
                fillers = fillers_by_run[h]
                k = 0
                for mt in range(TT):
                    if late_dmas and h < 2:
                        late_dmas.pop(0)()
                    if mt + 3 < TT:
                        bias_fetch(h, mt + 3,
                                   nc.sync if mt % 2 else nc.gpsimd)
                    if h + 1 < NH_L and mt >= 5:
                        bias_fetch(h + 1, mt - 5,
                                   nc.gpsimd if mt % 2 else nc.sync)
                    scores_step(h, mt)
                    want = (mt + 1) * len(fillers) // TT
                    while k < want:
                        fillers[k]()
                        k += 1
            # tail: head 7 attn.v (its av tiles borrow the idle scores
            # PSUM ring) + yB with drains alternating DVE/ACT and output
            # DMAs alternating SP/Pool so everything pipelines
            attn_v_A(7, 0, borrow_sc=True)
            attn_v_A(7, 1, borrow_sc=True)
            attn_v_B(7, 0)
            attn_v_B(7, 1)
            for n, (oc, qc) in enumerate(
                (oc, qc) for oc in range(FT) for qc in range(QC)
            ):
                y_unit(oc, qc, 1, act_copy=(n % 2 == 0),
                       borrow_sc=(n % 2 == 1))

    _split_waits(nc)
    _prog = nc
    return nc


def _in_maps(x_q, x_kv, bias, Wq, Wk, Wv, Wo):
    import ml_dtypes

    bf16 = ml_dtypes.bfloat16
    fp8 = ml_dtypes.float8_e4m3

    def cvt(a):
        return np.ascontiguousarray(a).astype(bf16)

    def pair8(a):
        # fp8 (value, residual) pair along a new leading axis
        a = np.ascontiguousarray(a).astype(np.float32)
        v8 = a.astype(fp8)
        r8 = (a - v8.astype(np.float32)).astype(fp8)
        return np.stack([v8, r8])

    def _bias_pack(bh):
        # [NH_L, S(q), S(k)] -> transposed tiles + fp8 residual pairs:
        # out[h, mt, 0] = fp8(biasT[k-block mt]), out[h, mt, 1] = fp8(residual)
        bT = np.ascontiguousarray(bh.swapaxes(1, 2)).astype(np.float32)
        b8 = bT.astype(fp8)
        r8 = (bT - b8.astype(np.float32)).astype(fp8)
        out = np.empty((NH_L, TT, 2, P, S), fp8)
        for mt in range(TT):
            out[:, mt, 0] = b8[:, mt * P : (mt + 1) * P, :]
            out[:, mt, 1] = r8[:, mt * P : (mt + 1) * P, :]
        return out

    maps = []
    for c in range(N_CORES):
        b, g = c // 2, c % 2
        hd = slice(g * JL, (g + 1) * JL)
        hs = slice(g * NH_L, (g + 1) * NH_L)
        maps.append(
            {
                "xqT": pair8(x_q[b].T),
                "xkvT": pair8(x_kv[b].T),
                "wqT": pair8(Wq[hd, :].T * 8.0),
                "wkT": pair8(Wk[hd, :].T * 64.0),
                "wvT": pair8(Wv[hd, :].T * 64.0),
                "woT": cvt(Wo[:, hd].T),
                "biasT": _bias_pack(bias[b, hs]),
            }
        )
    return maps


def _postprocess(results, x_q):
    y = np.empty((B, S, H), np.float32)
    for b in range(B):
        acc = np.zeros((H, S), np.float32)
        for c in (2 * b, 2 * b + 1):
            acc += results[c]["yA"].astype(np.float32)
            acc += results[c]["yB"].astype(np.float32)
        y[b] = x_q[b].astype(np.float32) + acc.T
    return y


def kernel(x_q, x_kv, bias, Wq, Wk, Wv, Wo):
    x_q = np.asarray(x_q)
    nc = _build()
    maps = _in_maps(x_q, np.asarray(x_kv), np.asarray(bias), np.asarray(Wq),
                    np.asarray(Wk), np.asarray(Wv), np.asarray(Wo))
    from concourse.bass_utils import run_bass_kernel_spmd

    res = run_bass_kernel_spmd(nc, maps, list(range(N_CORES)))
    return _postprocess(res.results, x_q)
